# revision 1
# baseline (speedup 1.0000x reference)
"""Multi-head causal attention (bs=4, L=2048, d_model=512, 8 heads x 64) on 8
Trainium2 NeuronCores.

Sharding: core c = (batch b = c//2, head-group hg = c%2); each core computes 4
heads of one batch over the full sequence. Host pre-transposes activations and
weight slices so every device matmul has its contraction dim on partitions;
device returns the transposed partial output projection; host sums the two
head-group partials per batch, transposes back and adds the (folded) biases.
"""

import numpy as np

import concourse.bacc as bacc
import concourse.mybir as mybir
import concourse.tile as tile
from concourse.bass_utils import run_bass_kernel_spmd

F32 = mybir.dt.float32
F32R = mybir.dt.float32r
F16 = mybir.dt.float16
AF = mybir.ActivationFunctionType

L = 2048          # sequence length
D = 512           # model dim
HD = 256          # head-group output dim (4 heads x 64)
DK = 64           # head dim
NH = 4            # heads per core
P = 128
IB = 512          # query block (i) width
NIB = L // IB     # 4 query blocks
NKT = D // P      # 4 contraction tiles over model dim
NJT = L // P      # 16 key tiles
SCALE = 1.0 / 8.0  # 1/sqrt(DK)

GRP = 2           # score j-tiles per PSUM/exp group


def _build(use_f32r=True):
    nc = bacc.Bacc("TRN2", target_bir_lowering=False, debug=False,
                   enable_asserts=False)

    xT = nc.dram_tensor("xT", [D, L], F16, kind="ExternalInput")
    wq = nc.dram_tensor("wq", [D, HD], F16, kind="ExternalInput")
    wk = nc.dram_tensor("wk", [D, HD], F16, kind="ExternalInput")
    wv = nc.dram_tensor("wv", [D, HD], F16, kind="ExternalInput")
    wo = nc.dram_tensor("wo", [HD, D], F16, kind="ExternalInput")
    bq = nc.dram_tensor("bq", [HD], F32, kind="ExternalInput")
    bk = nc.dram_tensor("bk", [HD], F32, kind="ExternalInput")
    outT = nc.dram_tensor("outT", [D, L], F32, kind="ExternalOutput")

    def r(ap):
        return ap

    with tile.TileContext(nc) as tc:
        with (
            tc.tile_pool(name="w", bufs=1) as pool_w,
            tc.tile_pool(name="x", bufs=NKT) as pool_x,
            tc.tile_pool(name="qk", bufs=1) as pool_qk,
            tc.tile_pool(name="v", bufs=NJT) as pool_v,
            tc.tile_pool(name="at", bufs=3) as pool_at,
            tc.tile_pool(name="zc", bufs=2) as pool_zc,
            tc.tile_pool(name="nm", bufs=2) as pool_nm,
            tc.tile_pool(name="o", bufs=2) as pool_o,
            tc.tile_pool(name="ps", bufs=1, space="PSUM") as pool_ps,
            tc.tile_pool(name="pz", bufs=2, space="PSUM") as pool_pz,
            tc.tile_pool(name="pp", bufs=2, space="PSUM") as pool_pp,
        ):
            # ---- loads: small weight/bias tiles first (they gate the
            # first matmuls), then the big xT tiles ----
            wq_sb = pool_w.tile([P, NKT, HD], F16, tag="wq")
            wk_sb = pool_w.tile([P, NKT, HD], F16, tag="wk")
            wv_sb = pool_w.tile([P, NKT, HD], F16, tag="wv")
            wo_sb = pool_w.tile([P, HD // P, D], F16, tag="wo")
            bq_sb = pool_w.tile([P, HD // P], F32, tag="bq")
            bk_sb = pool_w.tile([P, HD // P], F32, tag="bk")
            nc.sync.dma_start(wq_sb[:], wq.ap().rearrange("(t p) n -> p t n", p=P))
            nc.sync.dma_start(wk_sb[:], wk.ap().rearrange("(t p) n -> p t n", p=P))
            nc.sync.dma_start(wv_sb[:], wv.ap().rearrange("(t p) n -> p t n", p=P))
            nc.sync.dma_start(wo_sb[:], wo.ap().rearrange("(t p) n -> p t n", p=P))
            nc.sync.dma_start(bq_sb[:], bq.ap().rearrange("(t p) -> p t", p=P))
            nc.sync.dma_start(bk_sb[:], bk.ap().rearrange("(t p) -> p t", p=P))

            xts = []
            for kt in range(NKT):
                xt = pool_x.tile([P, L], F16)
                nc.sync.dma_start(xt[:], xT.ap()[kt * P:(kt + 1) * P, :])
                xts.append(xt)

            # ---- q/k projections: qT/kT[d, i] per d-tile (2 heads each) ----
            qk_tiles = {}

            def emit_qk(name, w_sb, b_sb, dt):
                dst = pool_qk.tile([P, L], F16, tag=f"{name}{dt}",
                                   name=f"{name}{dt}")
                qk_tiles[(name, dt)] = dst
                for ic in range(NIB):
                    pp = pool_pp.tile([P, IB], F32, tag="pp", name="pp")
                    for kt in range(NKT):
                        nc.tensor.matmul(
                            pp[:],
                            lhsT=r(w_sb[:, kt, dt * P:(dt + 1) * P]),
                            rhs=r(xts[kt][:, ic * IB:(ic + 1) * IB]),
                            start=(kt == 0), stop=(kt == NKT - 1),
                        )
                    nc.vector.tensor_scalar_add(
                        dst[:, ic * IB:(ic + 1) * IB], pp[:],
                        b_sb[:, dt:dt + 1])

            # ---- v projection: natural layout [j, (h, 65)], col 64 == 1.0 ----
            vts = []

            def emit_v(jt0, jt1):
                for jt in range(jt0, jt1):
                    vt = pool_v.tile([P, NH, DK + 1], F16, tag="v", name="v")
                    pp = pool_pp.tile([P, HD], F32, tag="pp", name="pp")
                    for kt in range(NKT):
                        nc.tensor.matmul(
                            pp[:],
                            lhsT=r(xts[kt][:, jt * P:(jt + 1) * P]),
                            rhs=r(wv_sb[:, kt, :]),
                            start=(kt == 0), stop=(kt == NKT - 1),
                        )
                    nc.vector.tensor_copy(
                        vt[:, :, 0:DK],
                        pp[:].rearrange("p (h e) -> p h e", h=NH))
                    nc.gpsimd.memset(vt[:, :, DK:DK + 1], 1.0)
                    vts.append(vt)

            # ---- attention + output projection, per query block ----
            # Heads run in PAIRS (2 per qk d-tile).
            zcs = {}

            def attn_pair(ib, hp):
                zc = zcs[ib]
                if True:
                    qt = qk_tiles[("q", hp)]
                    kt_t = qk_tiles[("k", hp)]
                    nj = 4 * (ib + 1)
                    pszs = [pool_pz.tile([P, IB], F32, tag="pz",
                                         name=f"pz{par}")
                            for par in range(2)]
                    def emit_z(jbs, z_ats):
                        for s, jb in enumerate(jbs):
                            vflat = vts[jb][:].rearrange("p h e -> p (h e)")
                            for par in range(2):
                                vcol = (2 * hp + par) * (DK + 1)
                                nc.tensor.matmul(
                                    pszs[par][0:DK + 1, :],
                                    lhsT=vflat[:, vcol:vcol + DK + 1],
                                    rhs=z_ats[par][:, s, :],
                                    start=(jb == 0), stop=(jb == nj - 1),
                                )

                    prev = None  # (jbs, ats) one group behind: the z matmuls
                    # lag the scores by a group so the exp+mask latency never
                    # stalls the in-order PE stream
                    for g in range((nj + GRP - 1) // GRP):
                        jbs = [g * GRP + s for s in range(GRP)
                               if g * GRP + s < nj]
                        pss = {}
                        ats = {}
                        for par in range(2):
                            pss[par] = pool_ps.tile([P, GRP, IB], F32,
                                                    tag=f"ps{par}",
                                                    name=f"ps{par}")
                            ats[par] = pool_at.tile([P, GRP, IB], F16,
                                                    tag=f"at{par}",
                                                    name=f"at{par}")
                        for s, jb in enumerate(jbs):
                            for par in range(2):
                                drow = DK * par
                                nc.tensor.matmul(
                                    pss[par][:, s, :],
                                    lhsT=kt_t[drow:drow + DK,
                                              jb * P:(jb + 1) * P],
                                    rhs=qt[drow:drow + DK,
                                           ib * IB:(ib + 1) * IB],
                                    start=True, stop=True,
                                )
                        if prev is not None:
                            emit_z(*prev)
                        for par in range(2):
                            nc.scalar.activation(ats[par][:], pss[par][:],
                                                 AF.Exp, scale=SCALE)
                            # causal mask on diagonal groups: keep iff
                            # i - j - 128*(t0+s) >= 0
                            t0 = g * GRP - 4 * ib
                            if t0 + GRP > 0:
                                nc.gpsimd.affine_select(
                                    ats[par][:], ats[par][:],
                                    pattern=[[-P, GRP], [1, IB]],
                                    compare_op=mybir.AluOpType.is_ge,
                                    fill=0.0, base=-P * t0,
                                    channel_multiplier=-1,
                                )
                        prev = (jbs, ats)
                    emit_z(*prev)
                    # normalize: z / denom (den row = partition 64).
                    # The approx-reciprocal custom DVE op only works from
                    # SBUF at base partition 0, and partition_broadcast's
                    # Q7 cpu0 only reads partitions 0-15 -- so copy den out
                    # of PSUM, bounce it to partition 0 by DMA, recip
                    # there, then broadcast.
                    for par in range(2):
                        psz = pszs[par]
                        zsb = pool_nm.tile([P, IB], F32, tag="zsb",
                                           name="zsb")
                        rec = pool_nm.tile([P, IB], F32, tag="rec",
                                           name="rec")
                        bct = pool_nm.tile([P, IB], F32, tag="bct",
                                           name="bct")
                        # one copy moves z+den out of PSUM so the bank frees
                        # for the next head pair immediately
                        nc.vector.tensor_copy(zsb[0:DK + 1, :],
                                              psz[0:DK + 1, :])
                        rc2 = pool_nm.tile([P, IB], F32, tag="rc2",
                                           name="rc2")
                        nc.gpsimd.dma_start(rec[0:1, :], zsb[DK:DK + 1, :])
                        nc.vector.reciprocal_approx_fast(rc2[0:1, :],
                                                         rec[0:1, :])
                        nc.gpsimd.partition_broadcast(
                            bct[0:DK, :], rc2[0:1, :], channels=DK)
                        if par == 0:
                            nc.vector.tensor_mul(zc[hp][0:DK, :],
                                                 zsb[0:DK, :], bct[0:DK, :])
                        else:
                            # DVE lanes are partition-locked; shift the odd
                            # head's rows 0:64 -> 64:128 via an SBUF DMA hop
                            zn = pool_nm.tile([P, IB], F16, tag="zn",
                                              name="zn")
                            nc.vector.tensor_mul(zn[0:DK, :],
                                                 zsb[0:DK, :], bct[0:DK, :])
                            nc.gpsimd.dma_start(zc[hp][DK:P, :], zn[0:DK, :])

            def outproj(ib):
                zc = zcs[ib]
                for mt in range(D // P):
                    po = pool_pp.tile([P, IB], F32, tag="pp", name="pp")
                    for kt2 in range(HD // P):
                        nc.tensor.matmul(
                            po[:],
                            lhsT=r(wo_sb[:, kt2, mt * P:(mt + 1) * P]),
                            rhs=r(zc[kt2][:]),
                            start=(kt2 == 0), stop=(kt2 == HD // P - 1),
                        )
                    osb = pool_o.tile([P, IB], F32, tag="o", name="o")
                    nc.vector.tensor_copy(osb[:], po[:])
                    nc.sync.dma_start(
                        outT.ap()[mt * P:(mt + 1) * P, ib * IB:(ib + 1) * IB],
                        osb[:])

            def new_zc(ib):
                zcs[ib] = [pool_zc.tile([P, IB], F16, tag=f"zc{dt}",
                                        name=f"zc{dt}")
                           for dt in range(HD // P)]

            # ---- emission schedule: pull early attention work ahead so
            # ACT/DVE/GpSimd ramp while the PE still runs projections ----
            emit_qk("q", wq_sb, bq_sb, 0)
            emit_qk("k", wk_sb, bk_sb, 0)
            emit_v(0, 4)
            new_zc(0)
            attn_pair(0, 0)
            emit_qk("q", wq_sb, bq_sb, 1)
            emit_qk("k", wk_sb, bk_sb, 1)
            emit_v(4, NJT)
            attn_pair(0, 1)
            outproj(0)
            for ib in range(1, NIB):
                new_zc(ib)
                attn_pair(ib, 0)
                attn_pair(ib, 1)
                outproj(ib)

    nc.compile()
    return nc


_NC = None


def _get_nc():
    global _NC
    if _NC is None:
        _NC = _build()
    return _NC


def _in_maps(x, w_q, b_q, w_k, b_k, w_v, b_v, w_o, b_o):
    maps = []
    for b in range(4):
        xTb = np.ascontiguousarray(x[b].T.astype(np.float16))
        for hg in range(2):
            sl = slice(hg * HD, (hg + 1) * HD)
            maps.append({
                "xT": xTb,
                "wq": np.ascontiguousarray(w_q[sl].T.astype(np.float16)),
                "wk": np.ascontiguousarray(w_k[sl].T.astype(np.float16)),
                "wv": np.ascontiguousarray(w_v[sl].T.astype(np.float16)),
                "wo": np.ascontiguousarray(w_o[:, sl].T.astype(np.float16)),
                "bq": np.ascontiguousarray(b_q[sl].astype(np.float32)),
                "bk": np.ascontiguousarray(b_k[sl].astype(np.float32)),
            })
    return maps


def _combine(results, w_o, b_v, b_o):
    corr = (b_o + w_o @ b_v).astype(np.float32)  # fold v/out biases
    out = np.empty((4, L, D), dtype=np.float32)
    for b in range(4):
        acc = results[2 * b]["outT"] + results[2 * b + 1]["outT"]
        out[b] = acc.T + corr
    return out


def kernel(x, w_q, b_q, w_k, b_k, w_v, b_v, w_o, b_o):
    nc = _get_nc()
    maps = _in_maps(x, w_q, b_q, w_k, b_k, w_v, b_v, w_o, b_o)
    res = run_bass_kernel_spmd(nc, maps, core_ids=list(range(8)))
    return _combine(res.results, w_o, b_v, b_o)


def bench(x, w_q, b_q, w_k, b_k, w_v, b_v, w_o, b_o):
    """Run with NTFF tracing; returns (output, exec_time_ns)."""
    nc = _get_nc()
    maps = _in_maps(x, w_q, b_q, w_k, b_k, w_v, b_v, w_o, b_o)
    res = run_bass_kernel_spmd(nc, maps, core_ids=list(range(8)), trace=True)
    return _combine(res.results, w_o, b_v, b_o), res.exec_time_ns



# revision 4
# speedup vs baseline: 1.1840x; 1.1840x over previous
"""Multi-head causal attention (bs=4, L=2048, d_model=512, 8 heads x 64) on 8
Trainium2 NeuronCores.

Sharding: core c = (batch b = c//2, head-group hg = c%2); each core computes 4
heads of one batch over the full sequence. Host pre-transposes activations and
weight slices so every device matmul has its contraction dim on partitions;
device returns the transposed partial output projection; host sums the two
head-group partials per batch, transposes back and adds the (folded) biases.
"""

import numpy as np

import concourse.bacc as bacc
import concourse.mybir as mybir
import concourse.tile as tile
from concourse.bass_utils import run_bass_kernel_spmd

F32 = mybir.dt.float32
F32R = mybir.dt.float32r
F16 = mybir.dt.float16
AF = mybir.ActivationFunctionType

L = 2048          # sequence length
D = 512           # model dim
HD = 256          # head-group output dim (4 heads x 64)
DK = 64           # head dim
NH = 4            # heads per core
P = 128
IB = 512          # query block (i) width
NIB = L // IB     # 4 query blocks
NKT = D // P      # 4 contraction tiles over model dim
NJT = L // P      # 16 key tiles
SCALE = 1.0 / 8.0  # 1/sqrt(DK)

GRP = 2           # score j-tiles per PSUM/exp group


def _build(use_f32r=True):
    nc = bacc.Bacc("TRN2", target_bir_lowering=False, debug=False,
                   enable_asserts=False)

    xT = nc.dram_tensor("xT", [D, L], F16, kind="ExternalInput")
    wq = nc.dram_tensor("wq", [D, HD], F16, kind="ExternalInput")
    wk = nc.dram_tensor("wk", [D, HD], F16, kind="ExternalInput")
    wv = nc.dram_tensor("wv", [D, HD], F16, kind="ExternalInput")
    wo = nc.dram_tensor("wo", [HD, D], F16, kind="ExternalInput")
    bq = nc.dram_tensor("bq", [HD], F32, kind="ExternalInput")
    bk = nc.dram_tensor("bk", [HD], F32, kind="ExternalInput")
    outT = nc.dram_tensor("outT", [D, L], F32, kind="ExternalOutput")

    def r(ap):
        return ap

    with tile.TileContext(nc) as tc:
        with (
            tc.tile_pool(name="w", bufs=1) as pool_w,
            tc.tile_pool(name="x", bufs=NKT) as pool_x,
            tc.tile_pool(name="qk", bufs=1) as pool_qk,
            tc.tile_pool(name="v", bufs=NJT) as pool_v,
            tc.tile_pool(name="at", bufs=3) as pool_at,
            tc.tile_pool(name="zc", bufs=2) as pool_zc,
            tc.tile_pool(name="nm", bufs=2) as pool_nm,
            tc.tile_pool(name="o", bufs=2) as pool_o,
            tc.tile_pool(name="ps", bufs=1, space="PSUM") as pool_ps,
            tc.tile_pool(name="pz", bufs=2, space="PSUM") as pool_pz,
            tc.tile_pool(name="pp", bufs=2, space="PSUM") as pool_pp,
        ):
            # ---- loads: small weight/bias tiles first (they gate the
            # first matmuls), then the big xT tiles ----
            wq_sb = pool_w.tile([P, NKT, HD], F16, tag="wq")
            wk_sb = pool_w.tile([P, NKT, HD], F16, tag="wk")
            wv_sb = pool_w.tile([P, NKT, HD], F16, tag="wv")
            wo_sb = pool_w.tile([P, HD // P, D], F16, tag="wo")
            bq_sb = pool_w.tile([P, HD // P], F32, tag="bq")
            bk_sb = pool_w.tile([P, HD // P], F32, tag="bk")
            nc.sync.dma_start(wq_sb[:], wq.ap().rearrange("(t p) n -> p t n", p=P))
            nc.sync.dma_start(wk_sb[:], wk.ap().rearrange("(t p) n -> p t n", p=P))
            nc.sync.dma_start(wv_sb[:], wv.ap().rearrange("(t p) n -> p t n", p=P))
            nc.sync.dma_start(wo_sb[:], wo.ap().rearrange("(t p) n -> p t n", p=P))
            nc.sync.dma_start(bq_sb[:], bq.ap().rearrange("(t p) -> p t", p=P))
            nc.sync.dma_start(bk_sb[:], bk.ap().rearrange("(t p) -> p t", p=P))

            xts = []
            for kt in range(NKT):
                xt = pool_x.tile([P, L], F16)
                nc.sync.dma_start(xt[:], xT.ap()[kt * P:(kt + 1) * P, :])
                xts.append(xt)

            # ---- q/k projections: qT/kT[d, i] per d-tile (2 heads each) ----
            qk_tiles = {}

            def emit_qk(name, w_sb, b_sb, dt):
                dst = pool_qk.tile([P, L], F16, tag=f"{name}{dt}",
                                   name=f"{name}{dt}")
                qk_tiles[(name, dt)] = dst
                for ic in range(NIB):
                    pp = pool_pp.tile([P, IB], F32, tag="pp", name="pp")
                    for kt in range(NKT):
                        nc.tensor.matmul(
                            pp[:],
                            lhsT=r(w_sb[:, kt, dt * P:(dt + 1) * P]),
                            rhs=r(xts[kt][:, ic * IB:(ic + 1) * IB]),
                            start=(kt == 0), stop=(kt == NKT - 1),
                        )
                    nc.vector.tensor_scalar_add(
                        dst[:, ic * IB:(ic + 1) * IB], pp[:],
                        b_sb[:, dt:dt + 1])

            # ---- v projection: natural layout [j, (h, 65)], col 64 == 1.0 ----
            vts = []

            def emit_v(jt0, jt1):
                for jt in range(jt0, jt1):
                    vt = pool_v.tile([P, NH, DK + 1], F16, tag="v", name="v")
                    pp = pool_pp.tile([P, HD], F32, tag="pp", name="pp")
                    for kt in range(NKT):
                        nc.tensor.matmul(
                            pp[:],
                            lhsT=r(xts[kt][:, jt * P:(jt + 1) * P]),
                            rhs=r(wv_sb[:, kt, :]),
                            start=(kt == 0), stop=(kt == NKT - 1),
                        )
                    nc.vector.tensor_copy(
                        vt[:, :, 0:DK],
                        pp[:].rearrange("p (h e) -> p h e", h=NH))
                    nc.gpsimd.memset(vt[:, :, DK:DK + 1], 1.0)
                    vts.append(vt)

            # ---- attention + output projection, per query block ----
            # Heads run in PAIRS (2 per qk d-tile).
            zcs = {}

            def attn_pair(ib, hp):
                zc = zcs[ib]
                if True:
                    qt = qk_tiles[("q", hp)]
                    kt_t = qk_tiles[("k", hp)]
                    nj = 4 * (ib + 1)
                    pszs = [pool_pz.tile([P, IB], F32, tag="pz",
                                         name=f"pz{par}")
                            for par in range(2)]
                    def emit_z(jbs, z_ats):
                        for s, jb in enumerate(jbs):
                            vflat = vts[jb][:].rearrange("p h e -> p (h e)")
                            for par in range(2):
                                vcol = (2 * hp + par) * (DK + 1)
                                nc.tensor.matmul(
                                    pszs[par][0:DK + 1, :],
                                    lhsT=vflat[:, vcol:vcol + DK + 1],
                                    rhs=z_ats[par][:, s, :],
                                    start=(jb == 0), stop=(jb == nj - 1),
                                )

                    prev = None  # (jbs, ats) one group behind: the z matmuls
                    # lag the scores by a group so the exp+mask latency never
                    # stalls the in-order PE stream
                    for g in range((nj + GRP - 1) // GRP):
                        jbs = [g * GRP + s for s in range(GRP)
                               if g * GRP + s < nj]
                        pss = {}
                        ats = {}
                        for par in range(2):
                            pss[par] = pool_ps.tile([P, GRP, IB], F32,
                                                    tag=f"ps{par}",
                                                    name=f"ps{par}")
                            ats[par] = pool_at.tile([P, GRP, IB], F16,
                                                    tag=f"at{par}",
                                                    name=f"at{par}")
                        for s, jb in enumerate(jbs):
                            for par in range(2):
                                drow = DK * par
                                nc.tensor.matmul(
                                    pss[par][:, s, :],
                                    lhsT=kt_t[drow:drow + DK,
                                              jb * P:(jb + 1) * P],
                                    rhs=qt[drow:drow + DK,
                                           ib * IB:(ib + 1) * IB],
                                    start=True, stop=True,
                                )
                        if prev is not None:
                            emit_z(*prev)
                        t0 = g * GRP - 4 * ib
                        for par in range(2):
                            nc.scalar.activation(ats[par][:], pss[par][:],
                                                 AF.Exp, scale=SCALE)
                            # causal mask, fine-grained: for diagonal j-tile
                            # u (= t0+s), queries c < 128u are fully masked
                            # (memset) and the [128,128] block at c-128u needs
                            # the triangular select. Off-diagonal tiles need
                            # nothing.
                            for s, jb in enumerate(jbs):
                                u = t0 + s
                                if u < 0:
                                    continue
                                at_s = ats[par][:, s, :]
                                if u > 0:
                                    nc.gpsimd.memset(at_s[0:P, 0:P * u], 0.0)
                                nc.gpsimd.affine_select(
                                    at_s[0:P, P * u:P * (u + 1)],
                                    at_s[0:P, P * u:P * (u + 1)],
                                    pattern=[[1, P]],
                                    compare_op=mybir.AluOpType.is_ge,
                                    fill=0.0, base=0,
                                    channel_multiplier=-1,
                                )
                        prev = (jbs, ats)
                    emit_z(*prev)
                    # normalize: z / denom (den row = psum partition 64).
                    # DVE-copy the den row out of PSUM (partition-locked, so
                    # it lands at partition 64), DMA-broadcast it to
                    # partitions 0:64 with a 0-stride source AP, approx-recip
                    # there (custom DVE op needs SBUF @ base partition 0),
                    # then multiply straight out of PSUM into the zc tile.
                    for par in range(2):
                        psz = pszs[par]
                        den = pool_nm.tile([P, IB], F32, tag="den",
                                           name="den")
                        bct = pool_nm.tile([P, IB], F32, tag="bct",
                                           name="bct")
                        nc.vector.tensor_copy(den[DK:DK + 1, :],
                                              psz[DK:DK + 1, :])
                        nc.sync.dma_start(den[0:1, :], den[DK:DK + 1, :])
                        nc.vector.reciprocal_approx_fast(den[0:1, :],
                                                         den[0:1, :])
                        nc.gpsimd.partition_broadcast(
                            bct[0:DK, :], den[0:1, :], channels=DK)
                        if par == 0:
                            nc.vector.tensor_mul(zc[hp][0:DK, :],
                                                 psz[0:DK, :], bct[0:DK, :])
                        else:
                            # DVE lanes are partition-locked; shift the odd
                            # head's rows 0:64 -> 64:128 via an SBUF DMA hop
                            zn = pool_nm.tile([P, IB], F16, tag="zn",
                                              name="zn")
                            nc.vector.tensor_mul(zn[0:DK, :],
                                                 psz[0:DK, :], bct[0:DK, :])
                            nc.sync.dma_start(zc[hp][DK:P, :], zn[0:DK, :])

            def outproj(ib):
                zc = zcs[ib]
                for mt in range(D // P):
                    po = pool_pp.tile([P, IB], F32, tag="pp", name="pp")
                    for kt2 in range(HD // P):
                        nc.tensor.matmul(
                            po[:],
                            lhsT=r(wo_sb[:, kt2, mt * P:(mt + 1) * P]),
                            rhs=r(zc[kt2][:]),
                            start=(kt2 == 0), stop=(kt2 == HD // P - 1),
                        )
                    osb = pool_o.tile([P, IB], F32, tag="o", name="o")
                    nc.vector.tensor_copy(osb[:], po[:])
                    nc.sync.dma_start(
                        outT.ap()[mt * P:(mt + 1) * P, ib * IB:(ib + 1) * IB],
                        osb[:])

            def new_zc(ib):
                zcs[ib] = [pool_zc.tile([P, IB], F16, tag=f"zc{dt}",
                                        name=f"zc{dt}")
                           for dt in range(HD // P)]

            # ---- emission schedule: pull early attention work ahead so
            # ACT/DVE/GpSimd ramp while the PE still runs projections ----
            emit_qk("q", wq_sb, bq_sb, 0)
            emit_qk("k", wk_sb, bk_sb, 0)
            emit_v(0, 4)
            new_zc(0)
            attn_pair(0, 0)
            emit_qk("q", wq_sb, bq_sb, 1)
            emit_qk("k", wk_sb, bk_sb, 1)
            emit_v(4, NJT)
            attn_pair(0, 1)
            outproj(0)
            for ib in range(1, NIB):
                new_zc(ib)
                attn_pair(ib, 0)
                attn_pair(ib, 1)
                outproj(ib)

    nc.compile()
    return nc


_NC = None


def _get_nc():
    global _NC
    if _NC is None:
        _NC = _build()
    return _NC


def _in_maps(x, w_q, b_q, w_k, b_k, w_v, b_v, w_o, b_o):
    maps = []
    for b in range(4):
        xTb = np.ascontiguousarray(x[b].T.astype(np.float16))
        for hg in range(2):
            sl = slice(hg * HD, (hg + 1) * HD)
            maps.append({
                "xT": xTb,
                "wq": np.ascontiguousarray(w_q[sl].T.astype(np.float16)),
                "wk": np.ascontiguousarray(w_k[sl].T.astype(np.float16)),
                "wv": np.ascontiguousarray(w_v[sl].T.astype(np.float16)),
                "wo": np.ascontiguousarray(w_o[:, sl].T.astype(np.float16)),
                "bq": np.ascontiguousarray(b_q[sl].astype(np.float32)),
                "bk": np.ascontiguousarray(b_k[sl].astype(np.float32)),
            })
    return maps


def _combine(results, w_o, b_v, b_o):
    corr = (b_o + w_o @ b_v).astype(np.float32)  # fold v/out biases
    out = np.empty((4, L, D), dtype=np.float32)
    for b in range(4):
        acc = results[2 * b]["outT"] + results[2 * b + 1]["outT"]
        out[b] = acc.T + corr
    return out


def kernel(x, w_q, b_q, w_k, b_k, w_v, b_v, w_o, b_o):
    nc = _get_nc()
    maps = _in_maps(x, w_q, b_q, w_k, b_k, w_v, b_v, w_o, b_o)
    res = run_bass_kernel_spmd(nc, maps, core_ids=list(range(8)))
    return _combine(res.results, w_o, b_v, b_o)


def bench(x, w_q, b_q, w_k, b_k, w_v, b_v, w_o, b_o):
    """Run with NTFF tracing; returns (output, exec_time_ns)."""
    nc = _get_nc()
    maps = _in_maps(x, w_q, b_q, w_k, b_k, w_v, b_v, w_o, b_o)
    res = run_bass_kernel_spmd(nc, maps, core_ids=list(range(8)), trace=True)
    return _combine(res.results, w_o, b_v, b_o), res.exec_time_ns

